# revision 3
# baseline (speedup 1.0000x reference)
"""Trainium2 Bass kernel for ConvolutionalAttention (B=2,S=2048,E=1024,H=16,KS=3).

Reference:  Q,K,V = query @ W.T + b;  scores = QK^T/sqrt(Dh) per head;
cross-head conv1d (H->H channels, kernel 3) along the key axis; softmax over
keys; out = (weights @ V) merged heads @ Wo.T + bo.

Strategy (8 cores, head-parallel, conv folded into K):
  K_conv[ho][k,(hi,d)] = sum_dk conv_w[ho,hi,dk] * K[k+dk-1,(hi,d)]
  => scores_conv[ho] = Q_full @ K_conv[ho]^T   (E=1024-deep matmul, computed
  transposed as [k,q]).  Each core owns H/8 = 2 output heads for all (b,q):
    1. one pass over host-transposed query^T computes Q^T (->DRAM, bf16),
       K^T (->SBUF, zero-padded edge cols) and V[s,d] (->SBUF), sharing every
       loaded rhs tile between the three projections; K_conv for e-chunk c is
       formed on VectorE right after chunk c's K columns land (overlapped);
    2. per (b, head): QK_conv matmuls (bf16, q-chunk-paired stationaries)
       -> PSUM f32 -> Exp on ScalarE (bf16 out) -> PV matmuls against
       ones-augmented V so the softmax denominator lands in PSUM row 64 ->
       reciprocal -> K=1-matmul broadcast -> normalize (+bv, bf16 out).
       bv is exact post-softmax (weights sum to 1); conv_b cancels inside
       softmax; 1/sqrt(Dh) folded into Wq/bq on host;
    3. AllToAll (bf16) reshards (head-slice -> q-slice); final Wo projection
       of this core's 512 output rows (bf16 matmuls, f32 accum + bias).
"""
import numpy as np
import ml_dtypes

import concourse.bacc as bacc
import concourse.mybir as mybir
import concourse.tile as tile
from concourse.bass_utils import run_bass_kernel_spmd

B, S, E, H, KS = 2, 2048, 1024, 16, 3
DH = E // H                  # 64
N_CORES = 8
HPC = H // N_CORES           # 2 heads per core
BS = B * S                   # 4096
QSLICE = BS // N_CORES       # 512 output rows per core
NE = E // 128                # 8 contraction chunks
NSC = BS // 512              # 8 s-chunks in projection pass
NKT = S // 128               # 16 k-tiles per batch
NQC = S // 512               # 4 q-chunks per batch
VROW = DH + 1                # 65: head block in augmented V
KT_PAD_W = 2 * S + 4         # [z | b0:S | z z | b1:S | z]
_B_OFF = (1, S + 3)
_PAD_COLS = (0, S + 1, S + 2, 2 * S + 3)

F32 = mybir.dt.float32
BF16 = mybir.dt.bfloat16
AL = mybir.AluOpType
AF = mybir.ActivationFunctionType


def build_nc(n_cores=N_CORES, collective=True):
    nc = bacc.Bacc("TRN2", target_bir_lowering=False, debug=False,
                   num_devices=n_cores)
    # inputs (host-prepped layouts; see prep_in_maps)
    qTh = nc.dram_tensor("qTh", [E, BS], BF16, kind="ExternalInput")
    wq_p = nc.dram_tensor("wq_p", [128, NE * NE * 128], BF16, kind="ExternalInput")
    wk_p = nc.dram_tensor("wk_p", [128, NE * NE * 128], BF16, kind="ExternalInput")
    wv_p = nc.dram_tensor("wv_p", [128, NE * HPC * DH], BF16, kind="ExternalInput")
    wo_p = nc.dram_tensor("wo_p", [128, NE * E], BF16, kind="ExternalInput")
    bq = nc.dram_tensor("bq", [128, NE], F32, kind="ExternalInput")
    bk = nc.dram_tensor("bk", [128, NE], F32, kind="ExternalInput")
    bv = nc.dram_tensor("bv", [128, HPC], F32, kind="ExternalInput")
    bo = nc.dram_tensor("bo", [128, E], F32, kind="ExternalInput")
    wvec = nc.dram_tensor("wvec", [128, HPC * KS * NE], F32, kind="ExternalInput")
    out = nc.dram_tensor("out", [QSLICE, E], F32, kind="ExternalOutput")

    with tile.TileContext(nc) as tc:
        with (
            tc.tile_pool(name="dram", bufs=1, space="DRAM") as dram,
            tc.tile_pool(name="persist", bufs=1) as persist,
        ):
            qproj_dram = dram.tile([E, BS], BF16)
            kconv_dram = dram.tile([HPC, E, BS], BF16)
            a2a_in = dram.tile([N_CORES * 128, QSLICE], BF16)
            a2a_out = dram.tile([N_CORES * 128, QSLICE], BF16)

            # augmented V: cols = g*(HPC*VROW) + h*VROW + [0..63]=d, 64=ones
            # where g = b*NKT + kt is the global k-tile index (32 of them)
            v_sb = persist.tile([128, B * NKT * HPC * VROW], BF16)
            bv_sb = persist.tile([128, HPC], F32)
            wvec_sb = persist.tile([128, HPC * KS * NE], F32)
            ones_sb = persist.tile([1, DH], BF16)
            wo_sb = persist.tile([128, NE * E], BF16)
            bo_sb = persist.tile([128, E], F32)
            nc.sync.dma_start(bv_sb[:], bv[:, :])
            nc.sync.dma_start(wvec_sb[:], wvec[:, :])
            nc.sync.dma_start(wo_sb[:], wo_p[:, :])
            nc.sync.dma_start(bo_sb[:], bo[:, :])
            nc.vector.memset(ones_sb[:], 1.0)
            for g in range(B * NKT):
                for h in range(HPC):
                    c0 = g * HPC * VROW + h * VROW + DH
                    nc.vector.memset(v_sb[:, c0:c0 + 1], 1.0)

            # ---------------- phase 1: projections + K_conv ----------------
            with (
                tc.tile_pool(name="proj", bufs=1) as proj,
                tc.tile_pool(name="pw", bufs=2) as pw,
                tc.tile_pool(name="pevac", bufs=3) as pevac,
                tc.tile_pool(name="ppsum", bufs=3, space="PSUM") as ppsum,
                tc.tile_pool(name="vpsum", bufs=2, space="PSUM") as vpsum,
                tc.tile_pool(name="kcv", bufs=2) as kcv,
            ):
                qt_full = proj.tile([128, NE * BS], BF16, tag="qtfull")
                kt_pad = proj.tile([128, NE * KT_PAD_W], BF16, tag="ktpad")
                wv_sb = proj.tile([128, NE * HPC * DH], BF16, tag="wv")
                bq_sb = proj.tile([128, NE], F32, tag="bq")
                bk_sb = proj.tile([128, NE], F32, tag="bk")

                # prefetch first two weight stripes before the bulk q loads
                wqk_pre = []
                for et in range(2):
                    wq_sb = pw.tile([128, NE * 128], BF16, tag="wqs",
                                    name=f"wqp{et}")
                    wk_sb = pw.tile([128, NE * 128], BF16, tag="wks",
                                    name=f"wkp{et}")
                    nc.sync.dma_start(wq_sb[:], wq_p[:, et * E:(et + 1) * E])
                    nc.sync.dma_start(wk_sb[:], wk_p[:, et * E:(et + 1) * E])
                    wqk_pre.append((wq_sb, wk_sb))
                nc.sync.dma_start(bq_sb[:], bq[:, :])
                nc.sync.dma_start(bk_sb[:], bk[:, :])
                nc.sync.dma_start(wv_sb[:], wv_p[:, :])
                # batch-0 halves of every contraction chunk first, so the
                # first s-chunk's j-accumulation can start ~12us earlier
                for half in range(2):
                    for j in range(NE):
                        c0 = half * S
                        nc.sync.dma_start(
                            qt_full[:, j * BS + c0:j * BS + c0 + S],
                            qTh[j * 128:(j + 1) * 128, c0:c0 + S])
                for c in range(NE):
                    for pc in _PAD_COLS:
                        col = c * KT_PAD_W + pc
                        nc.vector.memset(kt_pad[:, col:col + 1], 0.0)

                # Q^T and K^T: for each e-tile stream the packed weight
                # stripe; s-chunks paired so each stationary covers 1024 cols
                for et in range(NE):
                    if et < 2:
                        wq_sb, wk_sb = wqk_pre[et]
                    else:
                        wq_sb = pw.tile([128, NE * 128], BF16, tag="wqs")
                        wk_sb = pw.tile([128, NE * 128], BF16, tag="wks")
                        nc.sync.dma_start(wq_sb[:], wq_p[:, et * E:(et + 1) * E])
                        nc.sync.dma_start(wk_sb[:], wk_p[:, et * E:(et + 1) * E])
                    for scp in range(NSC // 2):
                        scs = (2 * scp, 2 * scp + 1)
                        pqs = [ppsum.tile([128, 512], F32, tag="pq",
                                          name=f"pq{s_}") for s_ in scs]
                        pks = [ppsum.tile([128, 512], F32, tag="pk",
                                          name=f"pk{s_}") for s_ in scs]
                        for j in range(NE):
                            w_j = wq_sb[:, j * 128:(j + 1) * 128]
                            for i, sc in enumerate(scs):
                                rhs = qt_full[:, j * BS + sc * 512:
                                              j * BS + (sc + 1) * 512]
                                nc.tensor.matmul(pqs[i][:], w_j, rhs,
                                                 start=(j == 0),
                                                 stop=(j == NE - 1))
                        for j in range(NE):
                            w_j = wk_sb[:, j * 128:(j + 1) * 128]
                            for i, sc in enumerate(scs):
                                rhs = qt_full[:, j * BS + sc * 512:
                                              j * BS + (sc + 1) * 512]
                                nc.tensor.matmul(pks[i][:], w_j, rhs,
                                                 start=(j == 0),
                                                 stop=(j == NE - 1))
                        for i, sc in enumerate(scs):
                            b_i, sc_i = divmod(sc, NQC)
                            qe = pevac.tile([128, 512], BF16, tag="qevac")
                            nc.scalar.activation(qe[:], pqs[i][:], AF.Identity,
                                                 bias=bq_sb[:, et:et + 1],
                                                 scale=1.0)
                            nc.sync.dma_start(
                                qproj_dram[et * 128:(et + 1) * 128,
                                           sc * 512:(sc + 1) * 512], qe[:])
                            kcol = et * KT_PAD_W + _B_OFF[b_i] + sc_i * 512
                            nc.scalar.activation(kt_pad[:, kcol:kcol + 512],
                                                 pks[i][:], AF.Identity,
                                                 bias=bk_sb[:, et:et + 1],
                                                 scale=1.0)

                    # K_conv for this finished e-chunk on VectorE (overlaps
                    # the next e-tile's matmuls)
                    c = et
                    for h in range(HPC):
                        for b_i in range(B):
                            base = c * KT_PAD_W + _B_OFF[b_i] - 1
                            t0 = kcv.tile([128, S], BF16, tag="t0")
                            t1 = kcv.tile([128, S], BF16, tag="t1")

                            def wv_(dk):
                                col = (h * KS + dk) * NE + c
                                return wvec_sb[:, col:col + 1]

                            nc.vector.tensor_scalar(
                                t0[:], kt_pad[:, base:base + S],
                                wv_(0), None, AL.mult)
                            nc.vector.tensor_scalar(
                                t1[:], kt_pad[:, base + 1:base + 1 + S],
                                wv_(1), None, AL.mult)
                            nc.vector.tensor_tensor(t0[:], t0[:], t1[:], AL.add)
                            nc.vector.tensor_scalar(
                                t1[:], kt_pad[:, base + 2:base + 2 + S],
                                wv_(2), None, AL.mult)
                            nc.vector.tensor_tensor(t0[:], t0[:], t1[:], AL.add)
                            nc.sync.dma_start(
                                kconv_dram[h, c * 128:(c + 1) * 128,
                                           b_i * S:(b_i + 1) * S], t0[:])

                # V: lhsT = raw query^T tiles, rhs = packed Wv^T slice
                for g in range(B * NKT):           # g = s-tile = global k-tile
                    pv = vpsum.tile([128, HPC * DH], F32, tag="pv")
                    for j in range(NE):
                        lhsT = qt_full[:, j * BS + g * 128:j * BS + (g + 1) * 128]
                        nc.tensor.matmul(
                            pv[:], lhsT,
                            wv_sb[:, j * HPC * DH:(j + 1) * HPC * DH],
                            start=(j == 0), stop=(j == NE - 1))
                    for h in range(HPC):
                        c0 = g * HPC * VROW + h * VROW
                        nc.scalar.activation(v_sb[:, c0:c0 + DH],
                                             pv[:, h * DH:(h + 1) * DH], AF.Copy)

            # ---------------- phase 2: attention ----------------
            with (
                tc.tile_pool(name="attn", bufs=1) as attn,
                tc.tile_pool(name="kcs", bufs=2) as kcs,
                tc.tile_pool(name="esb", bufs=6) as esb,
                tc.tile_pool(name="norm", bufs=4) as norm,
                tc.tile_pool(name="qkpsum", bufs=3, space="PSUM") as qkpsum,
                tc.tile_pool(name="pvpsum", bufs=4, space="PSUM") as pvpsum,
                tc.tile_pool(name="ivpsum", bufs=1, space="PSUM") as ivpsum,
            ):
                qp_res_b = [attn.tile([128, NE * S], BF16, tag=f"qpres{b_i}",
                                      name=f"qpres{b_i}")
                            for b_i in range(B)]

                def load_qp(b_i):
                    for j in range(NE):
                        nc.sync.dma_start(
                            qp_res_b[b_i][:, j * S:(j + 1) * S],
                            qproj_dram[j * 128:(j + 1) * 128,
                                       b_i * S:(b_i + 1) * S])

                load_qp(0)
                for b_i in range(B):
                    qp_res = qp_res_b[b_i]
                    for h in range(HPC):
                        if b_i == 0 and h == 1:
                            load_qp(1)   # prefetch under batch-0 compute
                        pvs = [pvpsum.tile([VROW, 512], F32, tag="pvp",
                                           name=f"pv{qq}")
                               for qq in range(NQC)]
                        inv_sb = norm.tile([1, S], BF16, tag="inv")
                        for kg in range(NKT // 4):
                            kc_sbs = []
                            for j in range(NE):
                                kt_sb = kcs.tile([128, 512], BF16,
                                                 tag=f"kc{j}", name=f"kcs{j}")
                                nc.sync.dma_start(
                                    kt_sb[:],
                                    kconv_dram[h, j * 128:(j + 1) * 128,
                                               b_i * S + kg * 512:
                                               b_i * S + (kg + 1) * 512])
                                kc_sbs.append(kt_sb)
                            for t in range(4):
                                kt = kg * 4 + t
                                g = b_i * NKT + kt
                                c0 = g * HPC * VROW + h * VROW
                                # q-chunks paired per stationary k-tile
                                for qp_ in range(NQC // 2):
                                    qcs = (2 * qp_, 2 * qp_ + 1)
                                    pss = [qkpsum.tile([128, 512], F32,
                                                       tag="qk",
                                                       name=f"qk{qc}")
                                           for qc in qcs]
                                    for j in range(NE):
                                        lhsT = kc_sbs[j][:, t * 128:(t + 1) * 128]
                                        for i, qc in enumerate(qcs):
                                            nc.tensor.matmul(
                                                pss[i][:], lhsT,
                                                qp_res[:, j * S + qc * 512:
                                                       j * S + (qc + 1) * 512],
                                                start=(j == 0),
                                                stop=(j == NE - 1))
                                    for i, qc in enumerate(qcs):
                                        ex = esb.tile([128, 512], BF16,
                                                      tag="exp")
                                        nc.scalar.activation(ex[:], pss[i][:],
                                                             AF.Exp)
                                        nc.tensor.matmul(
                                            pvs[qc][:], v_sb[:, c0:c0 + VROW],
                                            ex[:], start=(kt == 0),
                                            stop=(kt == NKT - 1))
                        # normalize + bias, ship to a2a bounce
                        for qc in range(NQC):
                            with nc.allow_low_precision(
                                    reason="softmax denom bf16 bcast"):
                                nc.vector.reciprocal(
                                    inv_sb[0:1, qc * 512:(qc + 1) * 512],
                                    pvs[qc][DH:DH + 1, :])
                            pi = ivpsum.tile([DH, 512], F32, tag="iv")
                            nc.tensor.matmul(
                                pi[:], ones_sb[0:1, :],
                                inv_sb[0:1, qc * 512:(qc + 1) * 512],
                                start=True, stop=True)
                            ib = norm.tile([DH, 512], F32, tag="invbc")
                            nc.scalar.activation(ib[:], pi[:], AF.Copy)
                            ho = norm.tile([DH, 512], BF16, tag="ho")
                            nc.vector.tensor_tensor(ho[:], pvs[qc][0:DH, :],
                                                    ib[:], AL.mult)
                            nc.vector.tensor_scalar(
                                ho[:], ho[:], bv_sb[0:DH, h:h + 1], None, AL.add)
                            piece = b_i * NQC + qc
                            r0 = piece * 128 + h * DH
                            nc.sync.dma_start(a2a_in[r0:r0 + DH, :], ho[:])

            # ---------------- phase 3: exchange + output proj ----------------
            if collective:
                nc.gpsimd.collective_compute(
                    "AllToAll", AL.bypass,
                    replica_groups=[list(range(n_cores))],
                    ins=[a2a_in.opt()], outs=[a2a_out.opt()])
            else:
                nc.sync.dma_start(a2a_out[:, :], a2a_in[:, :])

            with (
                tc.tile_pool(name="fin", bufs=1) as fin,
                tc.tile_pool(name="fevac", bufs=3) as fevac,
                tc.tile_pool(name="fpsum", bufs=2, space="PSUM") as fpsum,
            ):
                go_sb = fin.tile([128, NE * QSLICE], BF16, tag="go")
                for e in range(NE):
                    nc.sync.dma_start(go_sb[:, e * QSLICE:(e + 1) * QSLICE],
                                      a2a_out[e * 128:(e + 1) * 128, :])
                for qt in range(QSLICE // 128):
                    for nh in range(E // 512):
                        pf = fpsum.tile([128, 512], F32, tag="pf")
                        for e in range(NE):
                            lhsT = go_sb[:, e * QSLICE + qt * 128:
                                         e * QSLICE + (qt + 1) * 128]
                            rhs = wo_sb[:, e * E + nh * 512:e * E + (nh + 1) * 512]
                            nc.tensor.matmul(pf[:], lhsT, rhs,
                                             start=(e == 0), stop=(e == NE - 1))
                        ot = fevac.tile([128, 512], F32, tag="ot")
                        nc.vector.tensor_tensor(
                            ot[:], pf[:], bo_sb[:, nh * 512:(nh + 1) * 512],
                            AL.add)
                        nc.sync.dma_start(
                            out[qt * 128:(qt + 1) * 128,
                                nh * 512:(nh + 1) * 512], ot[:])
    nc.compile()
    return nc


def prep_in_maps(query, Wq, bq, Wk, bk, Wv, bv, Wo, bo, conv_w, conv_b):
    """Host-side layout prep. conv_b is dropped: softmax(x+c) == softmax(x)."""
    del conv_b
    scale = 1.0 / np.sqrt(DH)
    qT = np.ascontiguousarray(query.reshape(BS, E).T)           # [E, BS]
    qTh = qT.astype(ml_dtypes.bfloat16)

    def pack_w(WT):  # [E_in, E_out] -> [128, NE*NE*128], stripe et is
        # [128, NE*128] with col (j*128+e) = WT[j*128+p, et*128+e]
        Wr = WT.reshape(NE, 128, NE, 128)          # [j, p, et, e]
        return np.ascontiguousarray(
            Wr.transpose(1, 2, 0, 3).reshape(128, NE * NE * 128))

    wq_p = pack_w((Wq.T * scale).astype(np.float32)).astype(ml_dtypes.bfloat16)
    wk_p = pack_w(Wk.T.astype(np.float32)).astype(ml_dtypes.bfloat16)
    # wo packed: stripe e is [128, E] with col eo = Wo.T[e*128+p, eo]
    wo_p = np.ascontiguousarray(
        Wo.T.reshape(NE, 128, E).transpose(1, 0, 2).reshape(128, NE * E)
    ).astype(ml_dtypes.bfloat16)
    bq_a = np.ascontiguousarray((bq * scale).reshape(NE, 128).T).astype(np.float32)
    bk_a = np.ascontiguousarray(bk.reshape(NE, 128).T).astype(np.float32)
    bo_a = np.tile(bo.astype(np.float32)[None, :], (128, 1))

    in_maps = []
    for c in range(N_CORES):
        heads = [HPC * c + h for h in range(HPC)]
        # Wv^T slice packed: [128, NE*HPC*DH], col block j -> Wv[e_g, j*128+p]
        wv_cols = np.concatenate(
            [Wv[ho * DH:(ho + 1) * DH, :] for ho in heads], axis=0)  # [128,E]
        wv_p = np.ascontiguousarray(
            wv_cols.T.reshape(NE, 128, HPC * DH).transpose(1, 0, 2)
            .reshape(128, NE * HPC * DH)).astype(ml_dtypes.bfloat16)
        bv_a = np.zeros((128, HPC), np.float32)
        for h, ho in enumerate(heads):
            bv_a[0:DH, h] = bv[ho * DH:(ho + 1) * DH]
        wvec = np.zeros((128, HPC * KS * NE), np.float32)
        for h, ho in enumerate(heads):
            for dk in range(KS):
                col_v = np.repeat(conv_w[ho, :, dk], DH)       # [E]
                for cc in range(NE):
                    wvec[:, (h * KS + dk) * NE + cc] = col_v[cc * 128:(cc + 1) * 128]
        in_maps.append({
            "qTh": qTh, "wq_p": wq_p, "wk_p": wk_p, "wv_p": wv_p,
            "wo_p": wo_p, "bq": bq_a, "bk": bk_a, "bv": bv_a,
            "bo": bo_a, "wvec": wvec,
        })
    return in_maps


_NC_CACHE = {}


def kernel(**inputs) -> np.ndarray:
    in_maps = prep_in_maps(**{k: np.asarray(v) for k, v in inputs.items()})
    if "nc" not in _NC_CACHE:
        _NC_CACHE["nc"] = build_nc()
    nc = _NC_CACHE["nc"]
    res = run_bass_kernel_spmd(nc, in_maps, list(range(N_CORES)))
    full = np.concatenate([res.results[c]["out"] for c in range(N_CORES)],
                          axis=0)
    return full.reshape(B, S, E).astype(np.float32)


# revision 6
# speedup vs baseline: 74.8000x; 74.8000x over previous
"""Trainium2 Bass kernel for ConvolutionalAttention (B=2,S=2048,E=1024,H=16,KS=3).

Reference:  Q,K,V = query @ W.T + b;  scores = QK^T/sqrt(Dh) per head;
cross-head conv1d (H->H channels, kernel 3) along the key axis; softmax over
keys; out = (weights @ V) merged heads @ Wo.T + bo.

Strategy (8 cores, head-parallel, conv folded into K):
  K_conv[ho][k,(hi,d)] = sum_dk conv_w[ho,hi,dk] * K[k+dk-1,(hi,d)]
  => scores_conv[ho] = Q_full @ K_conv[ho]^T   (E=1024-deep matmul, computed
  transposed as [k,q]).  Each core owns H/8 = 2 output heads for all (b,q):
    1. one pass over host-transposed query^T computes Q^T (->DRAM, bf16),
       K^T (->SBUF, zero-padded edge cols) and V[s,d] (->SBUF), sharing every
       loaded rhs tile between the three projections; K_conv for e-chunk c is
       formed on VectorE right after chunk c's K columns land (overlapped);
    2. per (b, head): QK_conv matmuls (bf16, q-chunk-paired stationaries)
       -> PSUM f32 -> Exp on ScalarE (bf16 out) -> PV matmuls against
       ones-augmented V so the softmax denominator lands in PSUM row 64 ->
       reciprocal -> K=1-matmul broadcast -> normalize (+bv, bf16 out).
       bv is exact post-softmax (weights sum to 1); conv_b cancels inside
       softmax; 1/sqrt(Dh) folded into Wq/bq on host;
    3. AllToAll (bf16) reshards (head-slice -> q-slice); final Wo projection
       of this core's 512 output rows (bf16 matmuls, f32 accum + bias).
"""
import numpy as np
import ml_dtypes

import concourse.bacc as bacc
import concourse.mybir as mybir
import concourse.tile as tile
from concourse.bass_utils import run_bass_kernel_spmd

B, S, E, H, KS = 2, 2048, 1024, 16, 3
DH = E // H                  # 64
N_CORES = 8
HPC = H // N_CORES           # 2 heads per core
BS = B * S                   # 4096
QSLICE = BS // N_CORES       # 512 output rows per core
NE = E // 128                # 8 contraction chunks
NSC = BS // 512              # 8 s-chunks in projection pass
NKT = S // 128               # 16 k-tiles per batch
NQC = S // 512               # 4 q-chunks per batch
VROW = DH + 1                # 65: head block in augmented V
KT_PAD_W = 2 * S + 4         # [z | b0:S | z z | b1:S | z]
_B_OFF = (1, S + 3)
_PAD_COLS = (0, S + 1, S + 2, 2 * S + 3)

F32 = mybir.dt.float32
BF16 = mybir.dt.bfloat16
AL = mybir.AluOpType
AF = mybir.ActivationFunctionType


def build_nc(n_cores=N_CORES, collective=True):
    nc = bacc.Bacc("TRN2", target_bir_lowering=False, debug=False,
                   num_devices=n_cores)
    # inputs (host-prepped layouts; see prep_in_maps)
    qTh = nc.dram_tensor("qTh", [E, BS], BF16, kind="ExternalInput")
    wq_p = nc.dram_tensor("wq_p", [128, NE * NE * 128], BF16, kind="ExternalInput")
    wk_p = nc.dram_tensor("wk_p", [128, NE * NE * 128], BF16, kind="ExternalInput")
    wv_p = nc.dram_tensor("wv_p", [128, NE * HPC * DH], BF16, kind="ExternalInput")
    wo_p = nc.dram_tensor("wo_p", [128, NE * E], BF16, kind="ExternalInput")
    bq = nc.dram_tensor("bq", [128, NE], F32, kind="ExternalInput")
    bk = nc.dram_tensor("bk", [128, NE], F32, kind="ExternalInput")
    bv = nc.dram_tensor("bv", [128, HPC], F32, kind="ExternalInput")
    bo = nc.dram_tensor("bo", [128, E], F32, kind="ExternalInput")
    wvec = nc.dram_tensor("wvec", [128, HPC * KS * NE], F32, kind="ExternalInput")
    out = nc.dram_tensor("out", [QSLICE, E], F32, kind="ExternalOutput")

    with tile.TileContext(nc) as tc:
        with (
            tc.tile_pool(name="dram", bufs=1, space="DRAM") as dram,
            tc.tile_pool(name="persist", bufs=1) as persist,
        ):
            qproj_dram = dram.tile([E, BS], BF16)
            kconv_dram = dram.tile([HPC, E, BS], BF16)
            a2a_in = dram.tile([N_CORES * 128, QSLICE], BF16)
            a2a_out = dram.tile([N_CORES * 128, QSLICE], BF16)

            # augmented V: cols = g*(HPC*VROW) + h*VROW + [0..63]=d, 64=ones
            # where g = b*NKT + kt is the global k-tile index (32 of them)
            v_sb = persist.tile([128, B * NKT * HPC * VROW], BF16)
            bv_sb = persist.tile([128, HPC], F32)
            wvec_sb = persist.tile([128, HPC * KS * NE], F32)
            ones_sb = persist.tile([1, DH], BF16)
            wo_sb = persist.tile([128, NE * E], BF16)
            bo_sb = persist.tile([128, E], F32)
            nc.sync.dma_start(bv_sb[:], bv[:, :])
            nc.sync.dma_start(wvec_sb[:], wvec[:, :])
            nc.sync.dma_start(wo_sb[:], wo_p[:, :])
            nc.sync.dma_start(bo_sb[:], bo[:, :])
            nc.vector.memset(ones_sb[:], 1.0)
            for g in range(B * NKT):
                for h in range(HPC):
                    c0 = g * HPC * VROW + h * VROW + DH
                    nc.vector.memset(v_sb[:, c0:c0 + 1], 1.0)

            # ---------------- phase 1: projections + K_conv ----------------
            with (
                tc.tile_pool(name="proj", bufs=1) as proj,
                tc.tile_pool(name="pw", bufs=2) as pw,
                tc.tile_pool(name="pevac", bufs=3) as pevac,
                tc.tile_pool(name="ppsum", bufs=3, space="PSUM") as ppsum,
                tc.tile_pool(name="vpsum", bufs=2, space="PSUM") as vpsum,
                tc.tile_pool(name="kcv", bufs=2) as kcv,
            ):
                qt_full = proj.tile([128, NE * BS], BF16, tag="qtfull")
                kt_pad = proj.tile([128, NE * KT_PAD_W], BF16, tag="ktpad")
                wv_sb = proj.tile([128, NE * HPC * DH], BF16, tag="wv")
                bq_sb = proj.tile([128, NE], F32, tag="bq")
                bk_sb = proj.tile([128, NE], F32, tag="bk")

                # prefetch first two weight stripes before the bulk q loads
                wqk_pre = []
                for et in range(2):
                    wq_sb = pw.tile([128, NE * 128], BF16, tag="wqs",
                                    name=f"wqp{et}")
                    wk_sb = pw.tile([128, NE * 128], BF16, tag="wks",
                                    name=f"wkp{et}")
                    nc.sync.dma_start(wq_sb[:], wq_p[:, et * E:(et + 1) * E])
                    nc.sync.dma_start(wk_sb[:], wk_p[:, et * E:(et + 1) * E])
                    wqk_pre.append((wq_sb, wk_sb))
                nc.sync.dma_start(bq_sb[:], bq[:, :])
                nc.sync.dma_start(bk_sb[:], bk[:, :])
                nc.sync.dma_start(wv_sb[:], wv_p[:, :])
                # batch-0 halves of every contraction chunk first, so the
                # first s-chunk's j-accumulation can start ~12us earlier
                for half in range(2):
                    for j in range(NE):
                        c0 = half * S
                        nc.sync.dma_start(
                            qt_full[:, j * BS + c0:j * BS + c0 + S],
                            qTh[j * 128:(j + 1) * 128, c0:c0 + S])
                for c in range(NE):
                    for pc in _PAD_COLS:
                        col = c * KT_PAD_W + pc
                        nc.vector.memset(kt_pad[:, col:col + 1], 0.0)

                # Q^T and K^T: for each e-tile stream the packed weight
                # stripe; s-chunks paired so each stationary covers 1024 cols
                for et in range(NE):
                    if et < 2:
                        wq_sb, wk_sb = wqk_pre[et]
                    else:
                        wq_sb = pw.tile([128, NE * 128], BF16, tag="wqs")
                        wk_sb = pw.tile([128, NE * 128], BF16, tag="wks")
                        nc.sync.dma_start(wq_sb[:], wq_p[:, et * E:(et + 1) * E])
                        nc.sync.dma_start(wk_sb[:], wk_p[:, et * E:(et + 1) * E])
                    for scp in range(NSC // 2):
                        scs = (2 * scp, 2 * scp + 1)
                        pqs = [ppsum.tile([128, 512], F32, tag="pq",
                                          name=f"pq{s_}") for s_ in scs]
                        pks = [ppsum.tile([128, 512], F32, tag="pk",
                                          name=f"pk{s_}") for s_ in scs]
                        for j in range(NE):
                            w_j = wq_sb[:, j * 128:(j + 1) * 128]
                            for i, sc in enumerate(scs):
                                rhs = qt_full[:, j * BS + sc * 512:
                                              j * BS + (sc + 1) * 512]
                                nc.tensor.matmul(pqs[i][:], w_j, rhs,
                                                 start=(j == 0),
                                                 stop=(j == NE - 1))
                        for j in range(NE):
                            w_j = wk_sb[:, j * 128:(j + 1) * 128]
                            for i, sc in enumerate(scs):
                                rhs = qt_full[:, j * BS + sc * 512:
                                              j * BS + (sc + 1) * 512]
                                nc.tensor.matmul(pks[i][:], w_j, rhs,
                                                 start=(j == 0),
                                                 stop=(j == NE - 1))
                        for i, sc in enumerate(scs):
                            b_i, sc_i = divmod(sc, NQC)
                            qe = pevac.tile([128, 512], BF16, tag="qevac")
                            nc.scalar.activation(qe[:], pqs[i][:], AF.Identity,
                                                 bias=bq_sb[:, et:et + 1],
                                                 scale=1.0)
                            nc.sync.dma_start(
                                qproj_dram[et * 128:(et + 1) * 128,
                                           sc * 512:(sc + 1) * 512], qe[:])
                            kcol = et * KT_PAD_W + _B_OFF[b_i] + sc_i * 512
                            nc.scalar.activation(kt_pad[:, kcol:kcol + 512],
                                                 pks[i][:], AF.Identity,
                                                 bias=bk_sb[:, et:et + 1],
                                                 scale=1.0)

                    # K_conv for this finished e-chunk on VectorE (overlaps
                    # the next e-tile's matmuls)
                    c = et
                    for h in range(HPC):
                        for b_i in range(B):
                            base = c * KT_PAD_W + _B_OFF[b_i] - 1
                            t0 = kcv.tile([128, S], BF16, tag="t0")
                            t1 = kcv.tile([128, S], BF16, tag="t1")

                            def wv_(dk):
                                col = (h * KS + dk) * NE + c
                                return wvec_sb[:, col:col + 1]

                            nc.vector.tensor_scalar(
                                t0[:], kt_pad[:, base:base + S],
                                wv_(0), None, AL.mult)
                            nc.vector.tensor_scalar(
                                t1[:], kt_pad[:, base + 1:base + 1 + S],
                                wv_(1), None, AL.mult)
                            nc.vector.tensor_tensor(t0[:], t0[:], t1[:], AL.add)
                            nc.vector.tensor_scalar(
                                t1[:], kt_pad[:, base + 2:base + 2 + S],
                                wv_(2), None, AL.mult)
                            nc.vector.tensor_tensor(t0[:], t0[:], t1[:], AL.add)
                            nc.sync.dma_start(
                                kconv_dram[h, c * 128:(c + 1) * 128,
                                           b_i * S:(b_i + 1) * S], t0[:])

                # V: lhsT = raw query^T tiles, rhs = packed Wv^T slice
                for g in range(B * NKT):           # g = s-tile = global k-tile
                    pv = vpsum.tile([128, HPC * DH], F32, tag="pv")
                    for j in range(NE):
                        lhsT = qt_full[:, j * BS + g * 128:j * BS + (g + 1) * 128]
                        nc.tensor.matmul(
                            pv[:], lhsT,
                            wv_sb[:, j * HPC * DH:(j + 1) * HPC * DH],
                            start=(j == 0), stop=(j == NE - 1))
                    for h in range(HPC):
                        c0 = g * HPC * VROW + h * VROW
                        nc.scalar.activation(v_sb[:, c0:c0 + DH],
                                             pv[:, h * DH:(h + 1) * DH], AF.Copy)

            # ---------------- phase 2: attention ----------------
            with (
                tc.tile_pool(name="attn", bufs=1) as attn,
                tc.tile_pool(name="kcs", bufs=2) as kcs,
                tc.tile_pool(name="esb", bufs=6) as esb,
                tc.tile_pool(name="norm", bufs=4) as norm,
                tc.tile_pool(name="qkpsum", bufs=3, space="PSUM") as qkpsum,
                tc.tile_pool(name="pvpsum", bufs=4, space="PSUM") as pvpsum,
                tc.tile_pool(name="ivpsum", bufs=1, space="PSUM") as ivpsum,
            ):
                qp_res_b = [attn.tile([128, NE * S], BF16, tag=f"qpres{b_i}",
                                      name=f"qpres{b_i}")
                            for b_i in range(B)]

                def load_qp(b_i, qcs):
                    # qc-chunked so the first q-pair's loads land first
                    for qc in qcs:
                        for j in range(NE):
                            nc.sync.dma_start(
                                qp_res_b[b_i][:, j * S + qc * 512:
                                              j * S + (qc + 1) * 512],
                                qproj_dram[j * 128:(j + 1) * 128,
                                           b_i * S + qc * 512:
                                           b_i * S + (qc + 1) * 512])

                def load_kcs(b_i, h, kg):
                    kc_sbs = []
                    for j in range(NE):
                        kt_sb = kcs.tile([128, 512], BF16,
                                         tag=f"kc{j}", name=f"kcs{j}")
                        nc.sync.dma_start(
                            kt_sb[:],
                            kconv_dram[h, j * 128:(j + 1) * 128,
                                       b_i * S + kg * 512:
                                       b_i * S + (kg + 1) * 512])
                        kc_sbs.append(kt_sb)
                    return kc_sbs

                load_qp(0, (0, 1))
                kcs_pre = load_kcs(0, 0, 0)
                load_qp(0, (2, 3))
                for b_i in range(B):
                    qp_res = qp_res_b[b_i]
                    for h in range(HPC):
                        if b_i == 0 and h == 1:
                            load_qp(1, range(NQC))  # prefetch under b0 compute
                        pvs = [pvpsum.tile([VROW, 512], F32, tag="pvp",
                                           name=f"pv{qq}")
                               for qq in range(NQC)]
                        inv_sb = norm.tile([1, S], BF16, tag="inv")
                        for kg in range(NKT // 4):
                            if b_i == 0 and h == 0 and kg == 0:
                                kc_sbs = kcs_pre
                            else:
                                kc_sbs = load_kcs(b_i, h, kg)
                            for t in range(4):
                                kt = kg * 4 + t
                                g = b_i * NKT + kt
                                c0 = g * HPC * VROW + h * VROW
                                # q-chunks paired per stationary k-tile
                                for qp_ in range(NQC // 2):
                                    qcs = (2 * qp_, 2 * qp_ + 1)
                                    pss = [qkpsum.tile([128, 512], F32,
                                                       tag="qk",
                                                       name=f"qk{qc}")
                                           for qc in qcs]
                                    for j in range(NE):
                                        lhsT = kc_sbs[j][:, t * 128:(t + 1) * 128]
                                        for i, qc in enumerate(qcs):
                                            nc.tensor.matmul(
                                                pss[i][:], lhsT,
                                                qp_res[:, j * S + qc * 512:
                                                       j * S + (qc + 1) * 512],
                                                start=(j == 0),
                                                stop=(j == NE - 1))
                                    for i, qc in enumerate(qcs):
                                        ex = esb.tile([128, 512], BF16,
                                                      tag="exp")
                                        nc.scalar.activation(ex[:], pss[i][:],
                                                             AF.Exp)
                                        nc.tensor.matmul(
                                            pvs[qc][:], v_sb[:, c0:c0 + VROW],
                                            ex[:], start=(kt == 0),
                                            stop=(kt == NKT - 1))
                        # normalize + bias, ship to a2a bounce
                        for qc in range(NQC):
                            with nc.allow_low_precision(
                                    reason="softmax denom bf16 bcast"):
                                nc.vector.reciprocal(
                                    inv_sb[0:1, qc * 512:(qc + 1) * 512],
                                    pvs[qc][DH:DH + 1, :])
                            pi = ivpsum.tile([DH, 512], F32, tag="iv")
                            nc.tensor.matmul(
                                pi[:], ones_sb[0:1, :],
                                inv_sb[0:1, qc * 512:(qc + 1) * 512],
                                start=True, stop=True)
                            ib = norm.tile([DH, 512], F32, tag="invbc")
                            nc.scalar.activation(ib[:], pi[:], AF.Copy)
                            ho = norm.tile([DH, 512], BF16, tag="ho")
                            nc.vector.tensor_tensor(ho[:], pvs[qc][0:DH, :],
                                                    ib[:], AL.mult)
                            nc.vector.tensor_scalar(
                                ho[:], ho[:], bv_sb[0:DH, h:h + 1], None, AL.add)
                            piece = b_i * NQC + qc
                            r0 = piece * 128 + h * DH
                            nc.sync.dma_start(a2a_in[r0:r0 + DH, :], ho[:])

            # ---------------- phase 3: exchange + output proj ----------------
            if collective:
                nc.gpsimd.collective_compute(
                    "AllToAll", AL.bypass,
                    replica_groups=[list(range(n_cores))],
                    ins=[a2a_in.opt()], outs=[a2a_out.opt()])
            else:
                nc.sync.dma_start(a2a_out[:, :], a2a_in[:, :])

            with (
                tc.tile_pool(name="fin", bufs=1) as fin,
                tc.tile_pool(name="fevac", bufs=3) as fevac,
                tc.tile_pool(name="fpsum", bufs=2, space="PSUM") as fpsum,
            ):
                go_sb = fin.tile([128, NE * QSLICE], BF16, tag="go")
                for e in range(NE):
                    nc.sync.dma_start(go_sb[:, e * QSLICE:(e + 1) * QSLICE],
                                      a2a_out[e * 128:(e + 1) * 128, :])
                for qt in range(QSLICE // 128):
                    for nh in range(E // 512):
                        pf = fpsum.tile([128, 512], F32, tag="pf")
                        for e in range(NE):
                            lhsT = go_sb[:, e * QSLICE + qt * 128:
                                         e * QSLICE + (qt + 1) * 128]
                            rhs = wo_sb[:, e * E + nh * 512:e * E + (nh + 1) * 512]
                            nc.tensor.matmul(pf[:], lhsT, rhs,
                                             start=(e == 0), stop=(e == NE - 1))
                        ot = fevac.tile([128, 512], F32, tag="ot")
                        nc.vector.tensor_tensor(
                            ot[:], pf[:], bo_sb[:, nh * 512:(nh + 1) * 512],
                            AL.add)
                        nc.sync.dma_start(
                            out[qt * 128:(qt + 1) * 128,
                                nh * 512:(nh + 1) * 512], ot[:])
    nc.compile()
    return nc


def prep_in_maps(query, Wq, bq, Wk, bk, Wv, bv, Wo, bo, conv_w, conv_b):
    """Host-side layout prep. conv_b is dropped: softmax(x+c) == softmax(x)."""
    del conv_b
    scale = 1.0 / np.sqrt(DH)
    qT = np.ascontiguousarray(query.reshape(BS, E).T)           # [E, BS]
    qTh = qT.astype(ml_dtypes.bfloat16)

    def pack_w(WT):  # [E_in, E_out] -> [128, NE*NE*128], stripe et is
        # [128, NE*128] with col (j*128+e) = WT[j*128+p, et*128+e]
        Wr = WT.reshape(NE, 128, NE, 128)          # [j, p, et, e]
        return np.ascontiguousarray(
            Wr.transpose(1, 2, 0, 3).reshape(128, NE * NE * 128))

    wq_p = pack_w((Wq.T * scale).astype(np.float32)).astype(ml_dtypes.bfloat16)
    wk_p = pack_w(Wk.T.astype(np.float32)).astype(ml_dtypes.bfloat16)
    # wo packed: stripe e is [128, E] with col eo = Wo.T[e*128+p, eo]
    wo_p = np.ascontiguousarray(
        Wo.T.reshape(NE, 128, E).transpose(1, 0, 2).reshape(128, NE * E)
    ).astype(ml_dtypes.bfloat16)
    bq_a = np.ascontiguousarray((bq * scale).reshape(NE, 128).T).astype(np.float32)
    bk_a = np.ascontiguousarray(bk.reshape(NE, 128).T).astype(np.float32)
    bo_a = np.tile(bo.astype(np.float32)[None, :], (128, 1))

    in_maps = []
    for c in range(N_CORES):
        heads = [HPC * c + h for h in range(HPC)]
        # Wv^T slice packed: [128, NE*HPC*DH], col block j -> Wv[e_g, j*128+p]
        wv_cols = np.concatenate(
            [Wv[ho * DH:(ho + 1) * DH, :] for ho in heads], axis=0)  # [128,E]
        wv_p = np.ascontiguousarray(
            wv_cols.T.reshape(NE, 128, HPC * DH).transpose(1, 0, 2)
            .reshape(128, NE * HPC * DH)).astype(ml_dtypes.bfloat16)
        bv_a = np.zeros((128, HPC), np.float32)
        for h, ho in enumerate(heads):
            bv_a[0:DH, h] = bv[ho * DH:(ho + 1) * DH]
        wvec = np.zeros((128, HPC * KS * NE), np.float32)
        for h, ho in enumerate(heads):
            for dk in range(KS):
                col_v = np.repeat(conv_w[ho, :, dk], DH)       # [E]
                for cc in range(NE):
                    wvec[:, (h * KS + dk) * NE + cc] = col_v[cc * 128:(cc + 1) * 128]
        in_maps.append({
            "qTh": qTh, "wq_p": wq_p, "wk_p": wk_p, "wv_p": wv_p,
            "wo_p": wo_p, "bq": bq_a, "bk": bk_a, "bv": bv_a,
            "bo": bo_a, "wvec": wvec,
        })
    return in_maps


_NC_CACHE = {}


def kernel(**inputs) -> np.ndarray:
    in_maps = prep_in_maps(**{k: np.asarray(v) for k, v in inputs.items()})
    if "nc" not in _NC_CACHE:
        _NC_CACHE["nc"] = build_nc()
    nc = _NC_CACHE["nc"]
    res = run_bass_kernel_spmd(nc, in_maps, list(range(N_CORES)))
    full = np.concatenate([res.results[c]["out"] for c in range(N_CORES)],
                          axis=0)
    return full.reshape(B, S, E).astype(np.float32)
